# revision 1
# baseline (speedup 1.0000x reference)
"""GCNConv (graph message passing) on 8 Trainium2 NeuronCores — Bass/Tile.

out = a + (a @ Wres + bres),  a = relu(segment_sum(edge_val * (xW+b)[edge_col],
edge_row)),  computed via the identity  agg_lin = (A@x) @ W + deg x b  so the
sparse part runs on raw x, and the residual is folded into the weights:
out = relu(agg_lin) @ (Wres + I) + bres.

Sharding: nodes (segment-sum destinations) are partitioned across the 8 cores
(12500 nodes each); x and the small dense weights are replicated; each core
processes exactly the edges whose destination lands in its shard (host-side
routing).

Per-core device algorithm (bf16 compute, fully transposed, features on
partitions):
  For each superblock of 4 destination blocks (128 dests each):
  Phase 1: gather the needed bf16 source rows with dma_gather (int16 indices
  into <=25000-row chunks of x, sources sorted ascending within each group for
  HBM locality) and accumulate psum[f, d] += xg_tile.T @ S over the block's
  128-edge tiles, where S[e, d] = onehot(dest_in_block(e)) * edge_val(e) is
  built on the vector engine from an iota constant with one two-op
  tensor_scalar (is_equal then mult).  PSUM accumulates across all source
  chunks of a block.
  Phase 2 (fused, same superblock): agg_sb = bf16(psum) [128, 512];
  psA = W.T @ agg_sb + b x deg (rank-1); aT = relu(psA) in bf16;
  psB = (Wres+I).T @ aT + bres x 1; outT slab = f32(psB), DMA'd out
  transposed [128 features, 12544 dests] per core; the host transposes +
  concatenates.
"""
import math
import numpy as np

import concourse.tile as tile
from concourse import bacc, mybir
from concourse.bass_utils import run_bass_kernel_spmd

F32 = mybir.dt.float32
BF16 = mybir.dt.bfloat16
I16 = mybir.dt.int16
AL = mybir.AluOpType
ACT = mybir.ActivationFunctionType
D = 128
P = 128
N_CORES = 8
CH = 25000        # x chunk rows (int16 gather indices => <= 32767)
SBW = 4           # destination blocks per superblock (one gather per chunk)


def _build(n_src, n_blocks, tpb, repeat=1):
    nsh_pad = n_blocks * P
    Q = math.ceil(n_src / CH)
    G = Q * n_blocks * tpb
    IC = G * 8
    sbs = [list(range(s, min(s + SBW, n_blocks))) for s in range(0, n_blocks, SBW)]

    nc = bacc.Bacc("TRN2", target_bir_lowering=False, debug=False,
                   num_swdge_queues=2)
    x = nc.dram_tensor("x", [n_src, D], F32, kind="ExternalInput")
    W = nc.dram_tensor("W", [D, D], BF16, kind="ExternalInput")
    WresI = nc.dram_tensor("WresI", [D, D], BF16, kind="ExternalInput")
    bvec = nc.dram_tensor("bvec", [1, D], BF16, kind="ExternalInput")
    bres = nc.dram_tensor("bres", [1, D], BF16, kind="ExternalInput")
    iotaf = nc.dram_tensor("iotaf", [P, P], F32, kind="ExternalInput")
    idx = nc.dram_tensor("idx", [P, IC], I16, kind="ExternalInput")
    darr = nc.dram_tensor("darr", [P, G], F32, kind="ExternalInput")
    varr = nc.dram_tensor("varr", [P, G], F32, kind="ExternalInput")
    deg = nc.dram_tensor("deg", [1, nsh_pad], BF16, kind="ExternalInput")
    outT = nc.dram_tensor("outT", [D, nsh_pad], F32, kind="ExternalOutput")

    with tile.TileContext(nc) as tc:
        with tc.tile_pool(name="const", bufs=1) as cp:
            W_sb = cp.tile([D, D], BF16)
            nc.sync.dma_start(W_sb[:], W.ap())
            WresI_sb = cp.tile([D, D], BF16)
            nc.sync.dma_start(WresI_sb[:], WresI.ap())
            b_sb = cp.tile([1, D], BF16)
            nc.sync.dma_start(b_sb[:], bvec.ap())
            bres_sb = cp.tile([1, D], BF16)
            nc.sync.dma_start(bres_sb[:], bres.ap())
            deg_sb = cp.tile([1, nsh_pad], BF16)
            nc.sync.dma_start(deg_sb[:], deg.ap())
            iota_f = cp.tile([P, P], F32)
            nc.sync.dma_start(iota_f[:], iotaf.ap())
            idx_sb = cp.tile([P, IC], I16)
            nc.sync.dma_start(idx_sb[:], idx.ap())
            d_sb = cp.tile([P, G], F32)
            nc.sync.dma_start(d_sb[:], darr.ap())
            v_sb = cp.tile([P, G], F32)
            nc.sync.dma_start(v_sb[:], varr.ap())
            ones_row = cp.tile([1, SBW * P], BF16)
            nc.vector.memset(ones_row[:], 1.0)

            for _rep in range(repeat):
                with (
                    tc.tile_pool(name="xg", bufs=Q + 1) as xg_pool,
                    tc.tile_pool(name="s", bufs=6) as s_pool,
                    tc.tile_pool(name="agg", bufs=2) as agg_pool,
                    tc.tile_pool(name="a", bufs=2) as a_pool,
                    tc.tile_pool(name="o", bufs=2) as o_pool,
                    tc.tile_pool(name="ps1", bufs=4, space="PSUM") as ps1,
                    tc.tile_pool(name="psA", bufs=2, space="PSUM") as psA_pool,
                    tc.tile_pool(name="psB", bufs=2, space="PSUM") as psB_pool,
                ):
                    gt = 0
                    gq = 0
                    cbase = 0
                    for sb in sbs:
                        nb = len(sb)
                        w = nb * P
                        s0 = sb[0] * P
                        nidx = nb * tpb * P
                        # ---- Phase 1: gather + one-hot-matmul segment sum
                        xgs = []
                        for q in range(Q):
                            xg = xg_pool.tile([P, nb * tpb * P], F32, tag="xg",
                                              name=f"xg{q}")
                            nc.gpsimd.dma_gather(
                                xg[:].rearrange("p (t f) -> p t f", f=P),
                                x.ap()[q * CH: min(n_src, (q + 1) * CH), :],
                                idx_sb[:, cbase: cbase + nidx // 16],
                                nidx, nidx, D,
                                single_packet=(nidx <= 1024),
                                queue_num=gq % 2,
                            )
                            gq += 1
                            cbase += nidx // 16
                            xgs.append(xg)
                        pss = [ps1.tile([P, P], F32, tag="ps", name=f"ps{j}")
                               for j in range(nb)]
                        for q in range(Q):
                            for j in range(nb):
                                for t in range(tpb):
                                    S = s_pool.tile([P, P], F32, name="S")
                                    nc.vector.tensor_scalar(
                                        S[:], iota_f[:],
                                        d_sb[:, gt:gt + 1], v_sb[:, gt:gt + 1],
                                        op0=AL.is_equal, op1=AL.mult,
                                    )
                                    e0 = (j * tpb + t) * P
                                    nc.tensor.matmul(
                                        out=pss[j][:],
                                        lhsT=xgs[q][:, e0:e0 + P],
                                        rhs=S[:],
                                        start=(q == 0 and t == 0),
                                        stop=(q == Q - 1 and t == tpb - 1),
                                    )
                                    gt += 1
                        # ---- Phase 2 (fused): dense head on this superblock
                        agg_sb = agg_pool.tile([P, SBW * P], BF16)
                        for j in range(nb):
                            nc.scalar.activation(agg_sb[:, j * P:(j + 1) * P],
                                                 pss[j][:], ACT.Copy)
                        psA = psA_pool.tile([P, SBW * P], F32)
                        nc.tensor.matmul(out=psA[:, :w], lhsT=W_sb[:],
                                         rhs=agg_sb[:, :w],
                                         start=True, stop=False)
                        nc.tensor.matmul(out=psA[:, :w], lhsT=b_sb[:1, :],
                                         rhs=deg_sb[:1, s0:s0 + w],
                                         start=False, stop=True)
                        a_t = a_pool.tile([P, SBW * P], BF16)
                        nc.scalar.activation(a_t[:, :w], psA[:, :w], ACT.Relu)
                        psB = psB_pool.tile([P, SBW * P], F32)
                        nc.tensor.matmul(out=psB[:, :w], lhsT=WresI_sb[:],
                                         rhs=a_t[:, :w], start=True, stop=False)
                        nc.tensor.matmul(out=psB[:, :w], lhsT=bres_sb[:1, :],
                                         rhs=ones_row[:1, :w],
                                         start=False, stop=True)
                        o_t = o_pool.tile([P, SBW * P], F32)
                        nc.vector.tensor_copy(o_t[:, :w], psB[:, :w])
                        nc.sync.dma_start(outT.ap()[:, s0:s0 + w], o_t[:, :w])

    nc.compile()
    return nc


def _prep(x, W, b, Wres, bres, edge_val, edge_row, edge_col):
    import ml_dtypes
    BF = ml_dtypes.bfloat16
    x_f = np.ascontiguousarray(np.asarray(x, np.float32))
    W_bf = np.ascontiguousarray(np.asarray(W, np.float32).astype(BF))
    WresI_bf = np.ascontiguousarray(
        (np.asarray(Wres, np.float32) + np.eye(D, dtype=np.float32)).astype(BF))
    b_bf = np.asarray(b, np.float32).astype(BF).reshape(1, D)
    bres_bf = np.asarray(bres, np.float32).astype(BF).reshape(1, D)
    edge_row = np.asarray(edge_row)
    edge_col = np.asarray(edge_col)
    edge_val = np.asarray(edge_val, np.float32)

    N = np.asarray(x).shape[0]
    Q = math.ceil(N / CH)
    nsh = math.ceil(N / N_CORES)
    n_blocks = math.ceil(nsh / P)
    nsh_pad = n_blocks * P
    n_groups = n_blocks * Q

    shards = []
    tpb = 1
    for c in range(N_CORES):
        lo = c * nsh
        hi = min(N, lo + nsh)
        m = (edge_row >= lo) & (edge_row < hi)
        r = (edge_row[m] - lo).astype(np.int64)
        ci = edge_col[m].astype(np.int64)
        v = edge_val[m]
        blk = r >> 7
        q = ci // CH
        counts = np.bincount(blk * Q + q, minlength=n_groups)
        tpb = max(tpb, int(math.ceil(counts.max() / P)))
        shards.append((r, ci, v, blk, q))

    G = Q * n_blocks * tpb
    IC = G * 8
    sbs = [list(range(s, min(s + SBW, n_blocks))) for s in range(0, n_blocks, SBW)]
    grp_tile0 = np.zeros((n_blocks, Q), np.int64)
    tcur = 0
    for sb in sbs:
        nb = len(sb)
        for q in range(Q):
            for j, k in enumerate(sb):
                grp_tile0[k, q] = tcur + j * tpb
            tcur += nb * tpb
    assert tcur == G

    iota_f = np.tile(np.arange(P, dtype=np.float32), (P, 1))

    in_maps = []
    for c in range(N_CORES):
        r, ci, v, blk, q = shards[c]
        gid = blk * Q + q
        # sort by group, then ascending source within group (HBM locality)
        order = np.lexsort((ci, gid))
        r, ci, v, blk, q, gid = (a[order] for a in (r, ci, v, blk, q, gid))
        starts = np.zeros(n_groups + 1, np.int64)
        np.cumsum(np.bincount(gid, minlength=n_groups), out=starts[1:])
        ranks = np.arange(len(r), dtype=np.int64) - starts[gid]
        slot = (grp_tile0[blk, q] + (ranks >> 7)) * P + (ranks & 127)

        idx16 = np.zeros(G * P, np.int16)
        d_flat = np.zeros(G * P, np.float32)
        v_flat = np.zeros(G * P, np.float32)
        idx16[slot] = (ci - q * CH).astype(np.int16)
        d_flat[slot] = (r & 127).astype(np.float32)
        v_flat[slot] = v
        idx_h = np.tile(np.ascontiguousarray(idx16.reshape(IC, 16).T), (8, 1))
        d_h = np.ascontiguousarray(d_flat.reshape(G, P).T)
        v_h = np.ascontiguousarray(v_flat.reshape(G, P).T)
        degv = np.zeros(nsh_pad, np.float32)
        degv[:nsh] += np.bincount(r, weights=v, minlength=nsh
                                  ).astype(np.float32)[:nsh]
        in_maps.append({
            "x": x_f, "W": W_bf, "WresI": WresI_bf, "bvec": b_bf,
            "bres": bres_bf, "iotaf": iota_f, "idx": idx_h, "darr": d_h,
            "varr": v_h, "deg": degv.astype(BF).reshape(1, nsh_pad),
        })
    meta = dict(N=N, nsh=nsh, n_blocks=n_blocks, nsh_pad=nsh_pad, tpb=tpb, Q=Q)
    return in_maps, meta


def kernel(x, W, b, Wres, bres, edge_val, edge_row, edge_col):
    in_maps, meta = _prep(x, W, b, Wres, bres, edge_val, edge_row, edge_col)
    nc = _build(np.asarray(x).shape[0], meta["n_blocks"], meta["tpb"])
    res = run_bass_kernel_spmd(nc, in_maps, core_ids=list(range(N_CORES)))
    N, nsh = meta["N"], meta["nsh"]
    out = np.empty((N, D), np.float32)
    for c in range(N_CORES):
        lo = c * nsh
        hi = min(N, lo + nsh)
        out[lo:hi] = res.results[c]["outT"].T[: hi - lo]
    return out

